# revision 18
# baseline (speedup 1.0000x reference)
"""MoE layer (top-2 of 8 experts) on 8 Trainium2 NeuronCores.

Strategy (self-contained; shapes hardcoded for B=4,T=1024,D=1024,E=8,K=2,H=4096):
  - Host: gate logits + top-2 + softmax (float64 for a stable selection).
  - Slot-cover dispatch: every core runs the SAME kernel with two
    fixed-capacity column segments (s1 >= s2).  Chip-wide that gives 8 slots
    of cap s1 and 8 of cap s2; a small DP assigns each expert a set of slots
    (e.g. a 1101-token expert takes two s1 slots on different cores, a
    1035-token expert takes one s1 + one s2).  This shrinks the per-core
    column count C = s1+s2 from max_e(load_e) (~1101) toward the perfect
    balance sum/8 = 1024 while keeping one compile-time kernel structure.
  - Device, SPMD over 8 cores: per segment a transposed-layout FFN
      hT = gelu(w1.T @ xT + b1)   [H, s]
      yT = w2.T @ hT + b2         [D, s]
    bf16 matmul inputs, f32 PSUM accumulation; outputs DMA'd as bf16.
    Weight tiles are stationary (lhsT); tokens stream as the moving operand,
    so the column capacity per segment is exact (no 128-quantization).
    All chunks are >=256 cols so LDWEIGHTS (~100ns) hides under the stream.
  - Host combine: weighted scatter-add of the slot outputs.
"""

import os

import numpy as np
import ml_dtypes

B, T, D = 4, 1024, 1024
E, K, H = 8, 2, 4 * 1024
N = B * T
P = 128
KD = D // P           # 8  k-tiles in GEMM1 / output d-tiles in GEMM2
MH = H // P           # 32 m-tiles in GEMM1 / k-tiles in GEMM2
BF16 = ml_dtypes.bfloat16
WARMUP_MM = 27

LAST_EXEC_TIME_NS = None
_cached_nc = {}


def _chunks(c, lead=False):
    """Split a segment's columns into PSUM-bank-sized (<=512 f32) slices.

    Near-equal so every chunk stays >=256 for any c>=256, keeping LDWEIGHTS
    hidden under the matmul stream.  With lead=True the first chunk is
    shrunk to the 256-col minimum: it is the kernel's first DMA dependency,
    and a smaller first transfer starts the PE stream sooner.
    """
    if lead and 512 < c <= 768:
        return [slice(0, 256), slice(256, c)]
    n = -(-c // 512)
    base = c // n
    rem = c - base * n
    sizes = [base + (1 if i < rem else 0) for i in range(n)]
    out, off = [], 0
    for s in sizes:
        out.append(slice(off, off + s))
        off += s
    return out


def _solve_slots(loads):
    """Pick segment caps (s1, s2) and an expert->slot assignment.

    8 slots of cap s1 and 8 of cap s2 (one of each per core).  Expert e
    claims j s1-slots + k s2-slots with j*s1 + k*s2 >= loads[e].
    Minimizes C = s1 + s2; among ties prefers the largest minimum chunk.
    Returns (s1, s2, combos) with combos[e] = (j, k).
    """
    loads = list(map(int, loads))
    maxload = max(loads)
    lb = max(512, -(-sum(loads) // 8))
    c0 = 2 * (-(-maxload // 2))          # always-feasible fallback: (1,1) each

    def feasible(s1, s2):
        # Pareto-minimal combos per expert, then DP over (used1, used2).
        combo_opts = []
        for n_e in loads:
            opts = []
            if n_e == 0:
                opts.append((0, 0))
            else:
                for j in range(0, 9):
                    rest = n_e - j * s1
                    k = 0 if rest <= 0 else -(-rest // s2)
                    if k <= 8:
                        opts.append((j, k))
                        if rest <= 0:
                            break
            if not opts:
                return None
            combo_opts.append(opts)
        states = {(0, 0): []}
        for opts in combo_opts:
            nxt = {}
            for (u1, u2), hist in states.items():
                for (j, k) in opts:
                    v1, v2 = u1 + j, u2 + k
                    if v1 <= 8 and v2 <= 8 and (v1, v2) not in nxt:
                        nxt[(v1, v2)] = hist + [(j, k)]
            if not nxt:
                return None
            states = nxt
        return next(iter(states.values()))

    for C in range(lb, c0 + 1):
        best = None
        for s1 in range(-(-C // 2), C - 256 + 1):
            s2 = C - s1
            if s2 < 256:
                break
            combos = feasible(s1, s2)
            if combos is None:
                continue
            minchunk = min(min(sl.stop - sl.start for sl in _chunks(s))
                           for s in (s1, s2))
            if best is None or minchunk > best[0]:
                best = (minchunk, s1, s2, combos)
        if best is not None:
            return best[1], best[2], best[3]
    # unreachable (c0 config is feasible), but keep a hard fallback
    s = -(-maxload // 2)
    return s, s, [(1, 1) for _ in loads]


def _ensure_ntff_hook():
    """Register the axon NTFF profile hook if the image lacks antenv.axon_hooks."""
    import sys
    import types
    try:
        from antenv.axon_hooks import get_axon_ntff_profile_hook
        return get_axon_ntff_profile_hook() is not None
    except ImportError:
        pass
    try:
        import antenv
        from trn_agent_boot.trn_boot import _ntff_profile_via_ctypes
        mod = types.ModuleType("antenv.axon_hooks")
        holder = [None]
        mod.set_axon_ntff_profile_hook = lambda h: holder.__setitem__(0, h)
        mod.get_axon_ntff_profile_hook = lambda: holder[0]
        sys.modules["antenv.axon_hooks"] = mod
        antenv.axon_hooks = mod
        mod.set_axon_ntff_profile_hook(
            _ntff_profile_via_ctypes("/opt/axon/libaxon_pjrt.so"))
        return True
    except Exception:
        return False


def _build(s1, s2):
    import concourse.mybir as mybir
    import concourse.tile as tile
    from concourse import bacc

    nc = bacc.Bacc(None, target_bir_lowering=False)

    segs = [("A", s1), ("B", s2)]
    xs = {}
    w1p = {}
    b1p = {}
    w2p = {}
    b2p = {}
    outp = {}
    for sname, cap in segs:
        xs[sname] = [nc.declare_dram_parameter(
            f"x{sname}{i}", [P, KD, sl.stop - sl.start],
            mybir.dt.bfloat16, isOutput=False)
            for i, sl in enumerate(_chunks(cap, lead=sname == "A"))]
        w1p[sname] = nc.declare_dram_parameter(
            f"w1{sname}", [MH, P, KD, P], mybir.dt.bfloat16, isOutput=False)
        b1p[sname] = nc.declare_dram_parameter(
            f"b1{sname}", [P, MH], mybir.dt.float32, isOutput=False)
        w2p[sname] = nc.declare_dram_parameter(
            f"w2{sname}", [KD, P, MH, P], mybir.dt.bfloat16, isOutput=False)
        b2p[sname] = nc.declare_dram_parameter(
            f"b2{sname}", [P, KD], mybir.dt.float32, isOutput=False)
        outp[sname] = nc.declare_dram_parameter(
            f"out{sname}", [P, KD, cap], mybir.dt.bfloat16, isOutput=True)

    GELU = mybir.ActivationFunctionType.Gelu

    with tile.TileContext(nc) as tc, \
         tc.tile_pool(name="singles", bufs=1) as singles, \
         tc.tile_pool(name="w1poolA", bufs=5) as w1poolA, \
         tc.tile_pool(name="w1poolB", bufs=5) as w1poolB, \
         tc.tile_pool(name="w2poolA", bufs=2) as w2poolA, \
         tc.tile_pool(name="w2poolB", bufs=2) as w2poolB, \
         tc.tile_pool(name="ypool", bufs=3) as ypool, \
         tc.tile_pool(name="psum", bufs=4, space="PSUM") as psum_pool:

        w1pool = {"A": w1poolA, "B": w1poolB}
        w2pool = {"A": w2poolA, "B": w2poolB}

        # PE warm-up: dependency-free matmuls keep the TensorEngine busy
        # through the DMA ring-init + first-data window and release the HAM
        # clock gate, so the PE is warm when the real stream starts.
        warm_sb = singles.tile([P, 2 * P], mybir.dt.bfloat16)
        nc.vector.memset(warm_sb[:], 0.0)
        ps_warm = psum_pool.tile([P, 2 * P], mybir.dt.float32, name="ps_warm",
                                 tag="ps1")
        for _ in range(WARMUP_MM):
            nc.tensor.matmul(ps_warm[:], warm_sb[:, :P], warm_sb[:],
                             start=True, stop=True)

        # Segment-major schedule: all of A's GEMM1 first, then B's.  Only
        # segment A's x is on the startup critical path; B's inputs stream
        # in during A's ~30us of compute (sync queue drains in issue order,
        # so B's loads queue behind A's critical-path data).
        x_sbs = {}
        b1_sb = {}
        b2_sb = {}
        h_sb = {}
        for sname, cap in segs:
            h_sb[sname] = singles.tile([P, MH, cap], mybir.dt.bfloat16,
                                        name=f"h{sname}_sb")

        # GEMM1: hT[mo*128+p, c] = gelu(sum_k w1[k,:].T @ xT[k,:] + b1)
        NPRE = 4
        for sname, cap in segs:
            # interleave the first w1 tiles with the x chunks in issue order
            # so neither stream starves while the DMA rings ramp up
            pre = []
            w1_t0 = w1pool[sname].tile([P, KD, P], mybir.dt.bfloat16,
                                       name=f"w1{sname}_t")
            nc.sync.dma_start(out=w1_t0[:], in_=w1p[sname][0])
            pre.append(w1_t0)
            chs = _chunks(cap, lead=sname == "A")
            x_sbs[sname] = []
            b1_sb[sname] = singles.tile([P, MH], mybir.dt.float32,
                                         name=f"b1{sname}_sb")
            b2_sb[sname] = singles.tile([P, KD], mybir.dt.float32,
                                         name=f"b2{sname}_sb")
            for i, sl in enumerate(chs):
                x_sb = singles.tile([P, KD, sl.stop - sl.start],
                                    mybir.dt.bfloat16, name=f"x{sname}{i}")
                nc.sync.dma_start(out=x_sb[:], in_=xs[sname][i][:])
                x_sbs[sname].append(x_sb)
                w1_tn = w1pool[sname].tile([P, KD, P], mybir.dt.bfloat16,
                                           name=f"w1{sname}_t")
                nc.sync.dma_start(out=w1_tn[:], in_=w1p[sname][len(pre)])
                pre.append(w1_tn)
            nc.sync.dma_start(out=b1_sb[sname][:], in_=b1p[sname][:])
            nc.sync.dma_start(out=b2_sb[sname][:], in_=b2p[sname][:])
            while len(pre) < NPRE:
                w1_tn = w1pool[sname].tile([P, KD, P], mybir.dt.bfloat16,
                                           name=f"w1{sname}_t")
                nc.sync.dma_start(out=w1_tn[:], in_=w1p[sname][len(pre)])
                pre.append(w1_tn)
            for mo in range(MH):
                if mo < len(pre):
                    w1_t = pre[mo]
                else:
                    w1_t = w1pool[sname].tile([P, KD, P], mybir.dt.bfloat16,
                                              name=f"w1{sname}_t")
                    nc.sync.dma_start(out=w1_t[:], in_=w1p[sname][mo])
                for si, sl in enumerate(_chunks(cap, lead=sname == "A")):
                    ps1 = psum_pool.tile([P, sl.stop - sl.start],
                                         mybir.dt.float32, name="ps1")
                    for k in range(KD):
                        nc.tensor.matmul(ps1[:], w1_t[:, k, :],
                                         x_sbs[sname][si][:, k, :],
                                         start=(k == 0), stop=(k == KD - 1))
                    nc.scalar.activation(h_sb[sname][:, mo, sl], ps1[:], GELU,
                                         bias=b1_sb[sname][:, mo:mo + 1])

        # GEMM2: yT[do*128+p, c] = sum_k w2[k,:].T @ hT[k,:] + b2
        # B first so the kernel tail drains on A's smaller last chunk.
        for sname, cap in segs[::-1]:
            for do in range(KD):
                w2_t = w2pool[sname].tile([P, MH, P], mybir.dt.bfloat16,
                                          name=f"w2{sname}_t")
                nc.sync.dma_start(out=w2_t[:], in_=w2p[sname][do])
                for sl in _chunks(cap):
                    ps2 = psum_pool.tile([P, sl.stop - sl.start],
                                         mybir.dt.float32, name="ps2")
                    for k in range(MH):
                        nc.tensor.matmul(ps2[:], w2_t[:, k, :],
                                         h_sb[sname][:, k, sl],
                                         start=(k == 0), stop=(k == MH - 1))
                    y_sb = ypool.tile([P, sl.stop - sl.start],
                                      mybir.dt.bfloat16, name="y_sb")
                    nc.vector.tensor_scalar_add(y_sb[:], ps2[:],
                                                b2_sb[sname][:, do:do + 1])
                    nc.sync.dma_start(out=outp[sname][:, do, sl], in_=y_sb[:])

    nc.compile()
    return nc


def kernel(x, gate_w, gate_b, w1, b1, w2, b2):
    global LAST_EXEC_TIME_NS
    from concourse.bass_utils import run_bass_kernel_spmd

    x = np.asarray(x)
    xf = np.ascontiguousarray(x.reshape(N, D), dtype=np.float32)

    # --- Gate (host, float64 for a stable top-2 selection) ---
    logits = xf.astype(np.float64) @ np.asarray(gate_w).astype(np.float64)
    logits += np.asarray(gate_b).astype(np.float64)
    rows = np.arange(N)
    i1 = np.argmax(logits, axis=1)
    l1 = logits[rows, i1]
    tmp = logits.copy()
    tmp[rows, i1] = -np.inf
    i2 = np.argmax(tmp, axis=1)
    l2 = tmp[rows, i2]
    e2 = np.exp(l2 - l1)          # l1 >= l2
    wa = (1.0 / (1.0 + e2)).astype(np.float32)
    wb = (e2 / (1.0 + e2)).astype(np.float32)

    # --- Dispatch: per-expert token lists ---
    sels, wgts = [], []
    for e in range(E):
        sel = np.where((i1 == e) | (i2 == e))[0]
        wgt = np.where(i1[sel] == e, wa[sel], wb[sel])
        sels.append(sel)
        wgts.append(wgt)
    loads = [len(s) for s in sels]

    # --- Slot cover: segment caps + expert->slot assignment ---
    s1, s2, combos = _solve_slots(loads)
    caps = {"A": s1, "B": s2}

    # slot_expert[seg][core] = expert owning that slot (or -1 = unused)
    # slot_fill[seg][core] = (expert, start_idx, count) token range
    slot_expert = {"A": [-1] * E, "B": [-1] * E}
    slot_fill = {"A": [None] * E, "B": [None] * E}
    next_slot = {"A": 0, "B": 0}
    for e in range(E):
        j, k = combos[e]
        claimed = [("A", next_slot["A"] + i) for i in range(j)] + \
                  [("B", next_slot["B"] + i) for i in range(k)]
        next_slot["A"] += j
        next_slot["B"] += k
        off = 0
        for sname, slot in claimed:
            cnt = min(caps[sname], loads[e] - off)
            slot_expert[sname][slot] = e
            slot_fill[sname][slot] = (e, off, max(cnt, 0))
            off += max(cnt, 0)
        assert off >= loads[e], f"slot cover failed for expert {e}"

    # --- Per-core input maps ---
    w1a = np.asarray(w1, dtype=np.float32)
    b1a = np.asarray(b1, dtype=np.float32)
    w2a = np.asarray(w2, dtype=np.float32)
    b2a = np.asarray(b2, dtype=np.float32)
    wcache = {}

    def expert_weights(e):
        if e not in wcache:
            w1_r = np.ascontiguousarray(
                w1a[e].reshape(KD, P, MH, P).transpose(2, 1, 0, 3)).astype(BF16)
            w2_r = np.ascontiguousarray(
                w2a[e].reshape(MH, P, KD, P).transpose(2, 1, 0, 3)).astype(BF16)
            b1_r = np.ascontiguousarray(b1a[e].reshape(MH, P).T)
            b2_r = np.ascontiguousarray(b2a[e].reshape(KD, P).T)
            wcache[e] = (w1_r, b1_r, w2_r, b2_r)
        return wcache[e]

    in_maps = []
    for c in range(E):
        m = {}
        for sname in ("A", "B"):
            cap = caps[sname]
            fill = slot_fill[sname][c]
            xe = np.zeros((cap, D), dtype=np.float32)
            eid = 0
            if fill is not None:
                eid, off, cnt = fill
                xe[:cnt] = xf[sels[eid][off:off + cnt]]
            xT_r = xe.T.reshape(KD, P, cap).transpose(1, 0, 2).astype(BF16)
            w1_r, b1_r, w2_r, b2_r = expert_weights(eid)
            m[f"w1{sname}"] = w1_r
            m[f"b1{sname}"] = b1_r
            m[f"w2{sname}"] = w2_r
            m[f"b2{sname}"] = b2_r
            for i, sl in enumerate(_chunks(cap, lead=sname == "A")):
                m[f"x{sname}{i}"] = np.ascontiguousarray(xT_r[:, :, sl])
        in_maps.append(m)

    key = (s1, s2)
    if key not in _cached_nc:
        _cached_nc[key] = _build(s1, s2)
    nc = _cached_nc[key]

    trace = os.environ.get("MOE_KERNEL_PROFILE", "0") == "1"
    if trace:
        trace = _ensure_ntff_hook()
    res = None
    for attempt in range(3):
        try:
            res = run_bass_kernel_spmd(nc, in_maps, core_ids=list(range(E)),
                                       trace=trace and attempt == 0)
            break
        except Exception:
            # Device-unrecoverable NRT errors are transient here; retry with
            # a fresh PJRT client (last attempt re-raises).
            if attempt == 2:
                raise
            try:
                import jax
                jax.clear_caches()
                jax._src.api.clear_backends()
            except Exception:
                pass
    LAST_EXEC_TIME_NS = res.exec_time_ns

    # --- Combine (host) ---
    out_acc = np.zeros((N, D), dtype=np.float32)
    for sname in ("A", "B"):
        cap = caps[sname]
        for c in range(E):
            fill = slot_fill[sname][c]
            if fill is None or fill[2] == 0:
                continue
            e, off, cnt = fill
            yT = np.asarray(res.results[c][f"out{sname}"]).astype(np.float32)
            y = yT.transpose(1, 0, 2).reshape(D, cap).T    # [cap, D]
            toks = sels[e][off:off + cnt]
            out_acc[toks] += wgts[e][off:off + cnt, None] * y[:cnt]

    return out_acc.reshape(B, T, D)


# revision 19
# speedup vs baseline: 1.0206x; 1.0206x over previous
"""MoE layer (top-2 of 8 experts) on 8 Trainium2 NeuronCores.

Strategy (self-contained; shapes hardcoded for B=4,T=1024,D=1024,E=8,K=2,H=4096):
  - Host: gate logits + top-2 + softmax (float64 for a stable selection).
  - Slot-cover dispatch: every core runs the SAME kernel with two
    fixed-capacity column segments (s1 >= s2).  Chip-wide that gives 8 slots
    of cap s1 and 8 of cap s2; a small DP assigns each expert a set of slots
    (e.g. a 1101-token expert takes two s1 slots on different cores, a
    1035-token expert takes one s1 + one s2).  This shrinks the per-core
    column count C = s1+s2 from max_e(load_e) (~1101) toward the perfect
    balance sum/8 = 1024 while keeping one compile-time kernel structure.
  - Device, SPMD over 8 cores: per segment a transposed-layout FFN
      hT = gelu(w1.T @ xT + b1)   [H, s]
      yT = w2.T @ hT + b2         [D, s]
    bf16 matmul inputs, f32 PSUM accumulation; outputs DMA'd as bf16.
    Weight tiles are stationary (lhsT); tokens stream as the moving operand,
    so the column capacity per segment is exact (no 128-quantization).
    All chunks are >=256 cols so LDWEIGHTS (~100ns) hides under the stream.
  - Host combine: weighted scatter-add of the slot outputs.
"""

import os

import numpy as np
import ml_dtypes

B, T, D = 4, 1024, 1024
E, K, H = 8, 2, 4 * 1024
N = B * T
P = 128
KD = D // P           # 8  k-tiles in GEMM1 / output d-tiles in GEMM2
MH = H // P           # 32 m-tiles in GEMM1 / k-tiles in GEMM2
BF16 = ml_dtypes.bfloat16
WARMUP_MM = 28

LAST_EXEC_TIME_NS = None
_cached_nc = {}


def _chunks(c):
    """Split a segment's columns into PSUM-bank-sized (<=512 f32) slices.

    Near-equal so every chunk stays >=256 for any c>=256, keeping LDWEIGHTS
    (~117ns) hidden under the matmul stream.
    """
    n = -(-c // 512)
    base = c // n
    rem = c - base * n
    sizes = [base + (1 if i < rem else 0) for i in range(n)]
    out, off = [], 0
    for s in sizes:
        out.append(slice(off, off + s))
        off += s
    return out


def _solve_slots(loads):
    """Pick segment caps (s1, s2) and an expert->slot assignment.

    8 slots of cap s1 and 8 of cap s2 (one of each per core).  Expert e
    claims j s1-slots + k s2-slots with j*s1 + k*s2 >= loads[e].
    Minimizes C = s1 + s2; among ties prefers the largest minimum chunk.
    Returns (s1, s2, combos) with combos[e] = (j, k).
    """
    loads = list(map(int, loads))
    maxload = max(loads)
    lb = max(512, -(-sum(loads) // 8))
    c0 = 2 * (-(-maxload // 2))          # always-feasible fallback: (1,1) each

    def feasible(s1, s2):
        # Pareto-minimal combos per expert, then DP over (used1, used2).
        combo_opts = []
        for n_e in loads:
            opts = []
            if n_e == 0:
                opts.append((0, 0))
            else:
                for j in range(0, 9):
                    rest = n_e - j * s1
                    k = 0 if rest <= 0 else -(-rest // s2)
                    if k <= 8:
                        opts.append((j, k))
                        if rest <= 0:
                            break
            if not opts:
                return None
            combo_opts.append(opts)
        states = {(0, 0): []}
        for opts in combo_opts:
            nxt = {}
            for (u1, u2), hist in states.items():
                for (j, k) in opts:
                    v1, v2 = u1 + j, u2 + k
                    if v1 <= 8 and v2 <= 8 and (v1, v2) not in nxt:
                        nxt[(v1, v2)] = hist + [(j, k)]
            if not nxt:
                return None
            states = nxt
        return next(iter(states.values()))

    for C in range(lb, c0 + 1):
        best = None
        for s1 in range(-(-C // 2), C - 256 + 1):
            s2 = C - s1
            if s2 < 256:
                break
            combos = feasible(s1, s2)
            if combos is None:
                continue
            minchunk = min(min(sl.stop - sl.start for sl in _chunks(s))
                           for s in (s1, s2))
            if best is None or minchunk > best[0]:
                best = (minchunk, s1, s2, combos)
        if best is not None:
            return best[1], best[2], best[3]
    # unreachable (c0 config is feasible), but keep a hard fallback
    s = -(-maxload // 2)
    return s, s, [(1, 1) for _ in loads]


def _ensure_ntff_hook():
    """Register the axon NTFF profile hook if the image lacks antenv.axon_hooks."""
    import sys
    import types
    try:
        from antenv.axon_hooks import get_axon_ntff_profile_hook
        return get_axon_ntff_profile_hook() is not None
    except ImportError:
        pass
    try:
        import antenv
        from trn_agent_boot.trn_boot import _ntff_profile_via_ctypes
        mod = types.ModuleType("antenv.axon_hooks")
        holder = [None]
        mod.set_axon_ntff_profile_hook = lambda h: holder.__setitem__(0, h)
        mod.get_axon_ntff_profile_hook = lambda: holder[0]
        sys.modules["antenv.axon_hooks"] = mod
        antenv.axon_hooks = mod
        mod.set_axon_ntff_profile_hook(
            _ntff_profile_via_ctypes("/opt/axon/libaxon_pjrt.so"))
        return True
    except Exception:
        return False


def _build(s1, s2):
    import concourse.mybir as mybir
    import concourse.tile as tile
    from concourse import bacc

    nc = bacc.Bacc(None, target_bir_lowering=False)

    segs = [("A", s1), ("B", s2)]
    xs = {}
    w1p = {}
    b1p = {}
    w2p = {}
    b2p = {}
    outp = {}
    for sname, cap in segs:
        xs[sname] = [nc.declare_dram_parameter(
            f"x{sname}{i}", [P, KD, sl.stop - sl.start],
            mybir.dt.bfloat16, isOutput=False)
            for i, sl in enumerate(_chunks(cap))]
        w1p[sname] = nc.declare_dram_parameter(
            f"w1{sname}", [MH, P, KD, P], mybir.dt.bfloat16, isOutput=False)
        b1p[sname] = nc.declare_dram_parameter(
            f"b1{sname}", [P, MH], mybir.dt.float32, isOutput=False)
        w2p[sname] = nc.declare_dram_parameter(
            f"w2{sname}", [KD, P, MH, P], mybir.dt.bfloat16, isOutput=False)
        b2p[sname] = nc.declare_dram_parameter(
            f"b2{sname}", [P, KD], mybir.dt.float32, isOutput=False)
        outp[sname] = nc.declare_dram_parameter(
            f"out{sname}", [P, KD, cap], mybir.dt.bfloat16, isOutput=True)

    GELU = mybir.ActivationFunctionType.Gelu

    with tile.TileContext(nc) as tc, \
         tc.tile_pool(name="singles", bufs=1) as singles, \
         tc.tile_pool(name="w1poolA", bufs=5) as w1poolA, \
         tc.tile_pool(name="w1poolB", bufs=5) as w1poolB, \
         tc.tile_pool(name="w2poolA", bufs=2) as w2poolA, \
         tc.tile_pool(name="w2poolB", bufs=2) as w2poolB, \
         tc.tile_pool(name="ypool", bufs=3) as ypool, \
         tc.tile_pool(name="psum", bufs=4, space="PSUM") as psum_pool:

        w1pool = {"A": w1poolA, "B": w1poolB}
        w2pool = {"A": w2poolA, "B": w2poolB}

        # PE warm-up: dependency-free matmuls keep the TensorEngine busy
        # through the DMA ring-init + first-data window and release the HAM
        # clock gate, so the PE is warm when the real stream starts.
        warm_sb = singles.tile([P, 2 * P], mybir.dt.bfloat16)
        nc.vector.memset(warm_sb[:], 0.0)
        ps_warm = psum_pool.tile([P, 2 * P], mybir.dt.float32, name="ps_warm",
                                 tag="ps1")
        for _ in range(WARMUP_MM):
            nc.tensor.matmul(ps_warm[:], warm_sb[:, :P], warm_sb[:],
                             start=True, stop=True)

        # Segment-major schedule: all of A's GEMM1 first, then B's.  Only
        # segment A's x is on the startup critical path; B's inputs stream
        # in during A's ~30us of compute (sync queue drains in issue order,
        # so B's loads queue behind A's critical-path data).
        x_sbs = {}
        b1_sb = {}
        b2_sb = {}
        h_sb = {}
        for sname, cap in segs:
            h_sb[sname] = singles.tile([P, MH, cap], mybir.dt.bfloat16,
                                        name=f"h{sname}_sb")

        # GEMM1: hT[mo*128+p, c] = gelu(sum_k w1[k,:].T @ xT[k,:] + b1)
        NPRE = 4
        for sname, cap in segs:
            # interleave the first w1 tiles with the x chunks in issue order
            # so neither stream starves while the DMA rings ramp up
            pre = []
            w1_t0 = w1pool[sname].tile([P, KD, P], mybir.dt.bfloat16,
                                       name=f"w1{sname}_t")
            nc.sync.dma_start(out=w1_t0[:], in_=w1p[sname][0])
            pre.append(w1_t0)
            chs = _chunks(cap)
            x_sbs[sname] = []
            b1_sb[sname] = singles.tile([P, MH], mybir.dt.float32,
                                         name=f"b1{sname}_sb")
            b2_sb[sname] = singles.tile([P, KD], mybir.dt.float32,
                                         name=f"b2{sname}_sb")
            for i, sl in enumerate(chs):
                x_sb = singles.tile([P, KD, sl.stop - sl.start],
                                    mybir.dt.bfloat16, name=f"x{sname}{i}")
                nc.sync.dma_start(out=x_sb[:], in_=xs[sname][i][:])
                x_sbs[sname].append(x_sb)
                w1_tn = w1pool[sname].tile([P, KD, P], mybir.dt.bfloat16,
                                           name=f"w1{sname}_t")
                nc.sync.dma_start(out=w1_tn[:], in_=w1p[sname][len(pre)])
                pre.append(w1_tn)
            nc.sync.dma_start(out=b1_sb[sname][:], in_=b1p[sname][:])
            nc.sync.dma_start(out=b2_sb[sname][:], in_=b2p[sname][:])
            while len(pre) < NPRE:
                w1_tn = w1pool[sname].tile([P, KD, P], mybir.dt.bfloat16,
                                           name=f"w1{sname}_t")
                nc.sync.dma_start(out=w1_tn[:], in_=w1p[sname][len(pre)])
                pre.append(w1_tn)
            for mo in range(MH):
                if mo < len(pre):
                    w1_t = pre[mo]
                else:
                    w1_t = w1pool[sname].tile([P, KD, P], mybir.dt.bfloat16,
                                              name=f"w1{sname}_t")
                    nc.sync.dma_start(out=w1_t[:], in_=w1p[sname][mo])
                for si, sl in enumerate(_chunks(cap)):
                    ps1 = psum_pool.tile([P, sl.stop - sl.start],
                                         mybir.dt.float32, name="ps1")
                    for k in range(KD):
                        nc.tensor.matmul(ps1[:], w1_t[:, k, :],
                                         x_sbs[sname][si][:, k, :],
                                         start=(k == 0), stop=(k == KD - 1))
                    nc.scalar.activation(h_sb[sname][:, mo, sl], ps1[:], GELU,
                                         bias=b1_sb[sname][:, mo:mo + 1])

        # GEMM2: yT[do*128+p, c] = sum_k w2[k,:].T @ hT[k,:] + b2
        # B first so the kernel tail drains on A's smaller last chunk.
        for sname, cap in segs[::-1]:
            for do in range(KD):
                w2_t = w2pool[sname].tile([P, MH, P], mybir.dt.bfloat16,
                                          name=f"w2{sname}_t")
                nc.sync.dma_start(out=w2_t[:], in_=w2p[sname][do])
                for sl in _chunks(cap):
                    ps2 = psum_pool.tile([P, sl.stop - sl.start],
                                         mybir.dt.float32, name="ps2")
                    for k in range(MH):
                        nc.tensor.matmul(ps2[:], w2_t[:, k, :],
                                         h_sb[sname][:, k, sl],
                                         start=(k == 0), stop=(k == MH - 1))
                    y_sb = ypool.tile([P, sl.stop - sl.start],
                                      mybir.dt.bfloat16, name="y_sb")
                    nc.vector.tensor_scalar_add(y_sb[:], ps2[:],
                                                b2_sb[sname][:, do:do + 1])
                    nc.sync.dma_start(out=outp[sname][:, do, sl], in_=y_sb[:])

    nc.compile()
    return nc


def kernel(x, gate_w, gate_b, w1, b1, w2, b2):
    global LAST_EXEC_TIME_NS
    from concourse.bass_utils import run_bass_kernel_spmd

    x = np.asarray(x)
    xf = np.ascontiguousarray(x.reshape(N, D), dtype=np.float32)

    # --- Gate (host, float64 for a stable top-2 selection) ---
    logits = xf.astype(np.float64) @ np.asarray(gate_w).astype(np.float64)
    logits += np.asarray(gate_b).astype(np.float64)
    rows = np.arange(N)
    i1 = np.argmax(logits, axis=1)
    l1 = logits[rows, i1]
    tmp = logits.copy()
    tmp[rows, i1] = -np.inf
    i2 = np.argmax(tmp, axis=1)
    l2 = tmp[rows, i2]
    e2 = np.exp(l2 - l1)          # l1 >= l2
    wa = (1.0 / (1.0 + e2)).astype(np.float32)
    wb = (e2 / (1.0 + e2)).astype(np.float32)

    # --- Dispatch: per-expert token lists ---
    sels, wgts = [], []
    for e in range(E):
        sel = np.where((i1 == e) | (i2 == e))[0]
        wgt = np.where(i1[sel] == e, wa[sel], wb[sel])
        sels.append(sel)
        wgts.append(wgt)
    loads = [len(s) for s in sels]

    # --- Slot cover: segment caps + expert->slot assignment ---
    s1, s2, combos = _solve_slots(loads)
    caps = {"A": s1, "B": s2}

    # slot_expert[seg][core] = expert owning that slot (or -1 = unused)
    # slot_fill[seg][core] = (expert, start_idx, count) token range
    slot_expert = {"A": [-1] * E, "B": [-1] * E}
    slot_fill = {"A": [None] * E, "B": [None] * E}
    next_slot = {"A": 0, "B": 0}
    for e in range(E):
        j, k = combos[e]
        claimed = [("A", next_slot["A"] + i) for i in range(j)] + \
                  [("B", next_slot["B"] + i) for i in range(k)]
        next_slot["A"] += j
        next_slot["B"] += k
        off = 0
        for sname, slot in claimed:
            cnt = min(caps[sname], loads[e] - off)
            slot_expert[sname][slot] = e
            slot_fill[sname][slot] = (e, off, max(cnt, 0))
            off += max(cnt, 0)
        assert off >= loads[e], f"slot cover failed for expert {e}"

    # --- Per-core input maps ---
    w1a = np.asarray(w1, dtype=np.float32)
    b1a = np.asarray(b1, dtype=np.float32)
    w2a = np.asarray(w2, dtype=np.float32)
    b2a = np.asarray(b2, dtype=np.float32)
    wcache = {}

    def expert_weights(e):
        if e not in wcache:
            w1_r = np.ascontiguousarray(
                w1a[e].reshape(KD, P, MH, P).transpose(2, 1, 0, 3)).astype(BF16)
            w2_r = np.ascontiguousarray(
                w2a[e].reshape(MH, P, KD, P).transpose(2, 1, 0, 3)).astype(BF16)
            b1_r = np.ascontiguousarray(b1a[e].reshape(MH, P).T)
            b2_r = np.ascontiguousarray(b2a[e].reshape(KD, P).T)
            wcache[e] = (w1_r, b1_r, w2_r, b2_r)
        return wcache[e]

    in_maps = []
    for c in range(E):
        m = {}
        for sname in ("A", "B"):
            cap = caps[sname]
            fill = slot_fill[sname][c]
            xe = np.zeros((cap, D), dtype=np.float32)
            eid = 0
            if fill is not None:
                eid, off, cnt = fill
                xe[:cnt] = xf[sels[eid][off:off + cnt]]
            xT_r = xe.T.reshape(KD, P, cap).transpose(1, 0, 2).astype(BF16)
            w1_r, b1_r, w2_r, b2_r = expert_weights(eid)
            m[f"w1{sname}"] = w1_r
            m[f"b1{sname}"] = b1_r
            m[f"w2{sname}"] = w2_r
            m[f"b2{sname}"] = b2_r
            for i, sl in enumerate(_chunks(cap)):
                m[f"x{sname}{i}"] = np.ascontiguousarray(xT_r[:, :, sl])
        in_maps.append(m)

    key = (s1, s2)
    if key not in _cached_nc:
        _cached_nc[key] = _build(s1, s2)
    nc = _cached_nc[key]

    trace = os.environ.get("MOE_KERNEL_PROFILE", "0") == "1"
    if trace:
        trace = _ensure_ntff_hook()
    res = None
    for attempt in range(3):
        try:
            res = run_bass_kernel_spmd(nc, in_maps, core_ids=list(range(E)),
                                       trace=trace and attempt == 0)
            break
        except Exception:
            # Device-unrecoverable NRT errors are transient here; retry with
            # a fresh PJRT client (last attempt re-raises).
            if attempt == 2:
                raise
            try:
                import jax
                jax.clear_caches()
                jax._src.api.clear_backends()
            except Exception:
                pass
    LAST_EXEC_TIME_NS = res.exec_time_ns

    # --- Combine (host) ---
    out_acc = np.zeros((N, D), dtype=np.float32)
    for sname in ("A", "B"):
        cap = caps[sname]
        for c in range(E):
            fill = slot_fill[sname][c]
            if fill is None or fill[2] == 0:
                continue
            e, off, cnt = fill
            yT = np.asarray(res.results[c][f"out{sname}"]).astype(np.float32)
            y = yT.transpose(1, 0, 2).reshape(D, cap).T    # [cap, D]
            toks = sels[e][off:off + cnt]
            out_acc[toks] += wgts[e][off:off + cnt, None] * y[:cnt]

    return out_acc.reshape(B, T, D)


# revision 20
# speedup vs baseline: 1.0235x; 1.0028x over previous
"""MoE layer (top-2 of 8 experts) on 8 Trainium2 NeuronCores.

Strategy (self-contained; shapes hardcoded for B=4,T=1024,D=1024,E=8,K=2,H=4096):
  - Host: gate logits + top-2 + softmax (float64 for a stable selection).
  - Slot-cover dispatch: every core runs the SAME kernel with two
    fixed-capacity column segments (s1 >= s2).  Chip-wide that gives 8 slots
    of cap s1 and 8 of cap s2; a small DP assigns each expert a set of slots
    (e.g. a 1101-token expert takes two s1 slots on different cores, a
    1035-token expert takes one s1 + one s2).  This shrinks the per-core
    column count C = s1+s2 from max_e(load_e) (~1101) toward the perfect
    balance sum/8 = 1024 while keeping one compile-time kernel structure.
  - Device, SPMD over 8 cores: per segment a transposed-layout FFN
      hT = gelu(w1.T @ xT + b1)   [H, s]
      yT = w2.T @ hT + b2         [D, s]
    bf16 matmul inputs, f32 PSUM accumulation; outputs DMA'd as bf16.
    Weight tiles are stationary (lhsT); tokens stream as the moving operand,
    so the column capacity per segment is exact (no 128-quantization).
    All chunks are >=256 cols so LDWEIGHTS (~100ns) hides under the stream.
  - Host combine: weighted scatter-add of the slot outputs.
"""

import os

import numpy as np
import ml_dtypes

B, T, D = 4, 1024, 1024
E, K, H = 8, 2, 4 * 1024
N = B * T
P = 128
KD = D // P           # 8  k-tiles in GEMM1 / output d-tiles in GEMM2
MH = H // P           # 32 m-tiles in GEMM1 / k-tiles in GEMM2
BF16 = ml_dtypes.bfloat16
WARMUP_MM = 28

LAST_EXEC_TIME_NS = None
_cached_nc = {}


def _chunks(c):
    """Split a segment's columns into PSUM-bank-sized (<=512 f32) slices.

    Near-equal so every chunk stays >=256 for any c>=256, keeping LDWEIGHTS
    (~117ns) hidden under the matmul stream.
    """
    n = -(-c // 512)
    base = c // n
    rem = c - base * n
    sizes = [base + (1 if i < rem else 0) for i in range(n)]
    out, off = [], 0
    for s in sizes:
        out.append(slice(off, off + s))
        off += s
    return out


def _solve_slots(loads):
    """Pick segment caps (s1, s2) and an expert->slot assignment.

    8 slots of cap s1 and 8 of cap s2 (one of each per core).  Expert e
    claims j s1-slots + k s2-slots with j*s1 + k*s2 >= loads[e].
    Minimizes C = s1 + s2; among ties prefers the largest minimum chunk.
    Returns (s1, s2, combos) with combos[e] = (j, k).
    """
    import time as _time
    loads = list(map(int, loads))
    maxload = max(loads)
    lb = max(512, -(-sum(loads) // 8))
    c0 = 2 * (-(-maxload // 2))          # always-feasible fallback: (1,1) each
    deadline = _time.time() + 15.0

    def feasible(s1, s2):
        # Pareto-minimal combos per expert, then DP over (used1, used2).
        combo_opts = []
        for n_e in loads:
            opts = []
            if n_e == 0:
                opts.append((0, 0))
            else:
                for j in range(0, 9):
                    rest = n_e - j * s1
                    k = 0 if rest <= 0 else -(-rest // s2)
                    if k <= 8:
                        opts.append((j, k))
                        if rest <= 0:
                            break
            if not opts:
                return None
            combo_opts.append(opts)
        states = {(0, 0): []}
        for opts in combo_opts:
            nxt = {}
            for (u1, u2), hist in states.items():
                for (j, k) in opts:
                    v1, v2 = u1 + j, u2 + k
                    if v1 <= 8 and v2 <= 8 and (v1, v2) not in nxt:
                        nxt[(v1, v2)] = hist + [(j, k)]
            if not nxt:
                return None
            states = nxt
        return next(iter(states.values()))

    for C in range(lb, c0 + 1):
        best = None
        if _time.time() > deadline:
            break
        for s1 in range(-(-C // 2), C - 256 + 1):
            s2 = C - s1
            if s2 < 256:
                break
            combos = feasible(s1, s2)
            if combos is None:
                continue
            minchunk = min(min(sl.stop - sl.start for sl in _chunks(s))
                           for s in (s1, s2))
            if best is None or minchunk > best[0]:
                best = (minchunk, s1, s2, combos)
        if best is not None:
            return best[1], best[2], best[3]
    # timed out or exhausted: capacity fallback, always feasible
    s = max(256, -(-maxload // 2))
    return s, s, [(1, 1) for _ in loads]


def _ensure_ntff_hook():
    """Register the axon NTFF profile hook if the image lacks antenv.axon_hooks."""
    import sys
    import types
    try:
        from antenv.axon_hooks import get_axon_ntff_profile_hook
        return get_axon_ntff_profile_hook() is not None
    except ImportError:
        pass
    try:
        import antenv
        from trn_agent_boot.trn_boot import _ntff_profile_via_ctypes
        mod = types.ModuleType("antenv.axon_hooks")
        holder = [None]
        mod.set_axon_ntff_profile_hook = lambda h: holder.__setitem__(0, h)
        mod.get_axon_ntff_profile_hook = lambda: holder[0]
        sys.modules["antenv.axon_hooks"] = mod
        antenv.axon_hooks = mod
        mod.set_axon_ntff_profile_hook(
            _ntff_profile_via_ctypes("/opt/axon/libaxon_pjrt.so"))
        return True
    except Exception:
        return False


def _build(s1, s2):
    import concourse.mybir as mybir
    import concourse.tile as tile
    from concourse import bacc

    nc = bacc.Bacc(None, target_bir_lowering=False)

    segs = [("A", s1), ("B", s2)]
    xs = {}
    w1p = {}
    b1p = {}
    w2p = {}
    b2p = {}
    outp = {}
    for sname, cap in segs:
        xs[sname] = [nc.declare_dram_parameter(
            f"x{sname}{i}", [P, KD, sl.stop - sl.start],
            mybir.dt.bfloat16, isOutput=False)
            for i, sl in enumerate(_chunks(cap))]
        w1p[sname] = nc.declare_dram_parameter(
            f"w1{sname}", [MH, P, KD, P], mybir.dt.bfloat16, isOutput=False)
        b1p[sname] = nc.declare_dram_parameter(
            f"b1{sname}", [P, MH], mybir.dt.float32, isOutput=False)
        w2p[sname] = nc.declare_dram_parameter(
            f"w2{sname}", [KD, P, MH, P], mybir.dt.bfloat16, isOutput=False)
        b2p[sname] = nc.declare_dram_parameter(
            f"b2{sname}", [P, KD], mybir.dt.float32, isOutput=False)
        outp[sname] = nc.declare_dram_parameter(
            f"out{sname}", [P, KD, cap], mybir.dt.bfloat16, isOutput=True)

    GELU = mybir.ActivationFunctionType.Gelu

    with tile.TileContext(nc) as tc, \
         tc.tile_pool(name="singles", bufs=1) as singles, \
         tc.tile_pool(name="w1poolA", bufs=5) as w1poolA, \
         tc.tile_pool(name="w1poolB", bufs=5) as w1poolB, \
         tc.tile_pool(name="w2poolA", bufs=2) as w2poolA, \
         tc.tile_pool(name="w2poolB", bufs=2) as w2poolB, \
         tc.tile_pool(name="ypool", bufs=3) as ypool, \
         tc.tile_pool(name="psum", bufs=4, space="PSUM") as psum_pool:

        w1pool = {"A": w1poolA, "B": w1poolB}
        w2pool = {"A": w2poolA, "B": w2poolB}

        # PE warm-up: dependency-free matmuls keep the TensorEngine busy
        # through the DMA ring-init + first-data window and release the HAM
        # clock gate, so the PE is warm when the real stream starts.
        warm_sb = singles.tile([P, 2 * P], mybir.dt.bfloat16)
        nc.vector.memset(warm_sb[:], 0.0)
        ps_warm = psum_pool.tile([P, 2 * P], mybir.dt.float32, name="ps_warm",
                                 tag="ps1")
        for _ in range(WARMUP_MM):
            nc.tensor.matmul(ps_warm[:], warm_sb[:, :P], warm_sb[:],
                             start=True, stop=True)

        # Segment-major schedule: all of A's GEMM1 first, then B's.  Only
        # segment A's x is on the startup critical path; B's inputs stream
        # in during A's ~30us of compute (sync queue drains in issue order,
        # so B's loads queue behind A's critical-path data).
        x_sbs = {}
        b1_sb = {}
        b2_sb = {}
        h_sb = {}
        for sname, cap in segs:
            h_sb[sname] = singles.tile([P, MH, cap], mybir.dt.bfloat16,
                                        name=f"h{sname}_sb")

        # GEMM1: hT[mo*128+p, c] = gelu(sum_k w1[k,:].T @ xT[k,:] + b1)
        NPRE = 4
        for sname, cap in segs:
            # interleave the first w1 tiles with the x chunks in issue order
            # so neither stream starves while the DMA rings ramp up
            pre = []
            w1_t0 = w1pool[sname].tile([P, KD, P], mybir.dt.bfloat16,
                                       name=f"w1{sname}_t")
            nc.sync.dma_start(out=w1_t0[:], in_=w1p[sname][0])
            pre.append(w1_t0)
            chs = _chunks(cap)
            x_sbs[sname] = []
            b1_sb[sname] = singles.tile([P, MH], mybir.dt.float32,
                                         name=f"b1{sname}_sb")
            b2_sb[sname] = singles.tile([P, KD], mybir.dt.float32,
                                         name=f"b2{sname}_sb")
            for i, sl in enumerate(chs):
                x_sb = singles.tile([P, KD, sl.stop - sl.start],
                                    mybir.dt.bfloat16, name=f"x{sname}{i}")
                nc.sync.dma_start(out=x_sb[:], in_=xs[sname][i][:])
                x_sbs[sname].append(x_sb)
                w1_tn = w1pool[sname].tile([P, KD, P], mybir.dt.bfloat16,
                                           name=f"w1{sname}_t")
                nc.sync.dma_start(out=w1_tn[:], in_=w1p[sname][len(pre)])
                pre.append(w1_tn)
            nc.sync.dma_start(out=b1_sb[sname][:], in_=b1p[sname][:])
            nc.sync.dma_start(out=b2_sb[sname][:], in_=b2p[sname][:])
            while len(pre) < NPRE:
                w1_tn = w1pool[sname].tile([P, KD, P], mybir.dt.bfloat16,
                                           name=f"w1{sname}_t")
                nc.sync.dma_start(out=w1_tn[:], in_=w1p[sname][len(pre)])
                pre.append(w1_tn)
            for mo in range(MH):
                if mo < len(pre):
                    w1_t = pre[mo]
                else:
                    w1_t = w1pool[sname].tile([P, KD, P], mybir.dt.bfloat16,
                                              name=f"w1{sname}_t")
                    nc.sync.dma_start(out=w1_t[:], in_=w1p[sname][mo])
                for si, sl in enumerate(_chunks(cap)):
                    ps1 = psum_pool.tile([P, sl.stop - sl.start],
                                         mybir.dt.float32, name="ps1")
                    for k in range(KD):
                        nc.tensor.matmul(ps1[:], w1_t[:, k, :],
                                         x_sbs[sname][si][:, k, :],
                                         start=(k == 0), stop=(k == KD - 1))
                    nc.scalar.activation(h_sb[sname][:, mo, sl], ps1[:], GELU,
                                         bias=b1_sb[sname][:, mo:mo + 1])

        # GEMM2: yT[do*128+p, c] = sum_k w2[k,:].T @ hT[k,:] + b2
        # B first so the kernel tail drains on A's smaller last chunk.
        for sname, cap in segs[::-1]:
            for do in range(KD):
                w2_t = w2pool[sname].tile([P, MH, P], mybir.dt.bfloat16,
                                          name=f"w2{sname}_t")
                nc.sync.dma_start(out=w2_t[:], in_=w2p[sname][do])
                for sl in _chunks(cap):
                    ps2 = psum_pool.tile([P, sl.stop - sl.start],
                                         mybir.dt.float32, name="ps2")
                    for k in range(MH):
                        nc.tensor.matmul(ps2[:], w2_t[:, k, :],
                                         h_sb[sname][:, k, sl],
                                         start=(k == 0), stop=(k == MH - 1))
                    y_sb = ypool.tile([P, sl.stop - sl.start],
                                      mybir.dt.bfloat16, name="y_sb")
                    nc.vector.tensor_scalar_add(y_sb[:], ps2[:],
                                                b2_sb[sname][:, do:do + 1])
                    nc.sync.dma_start(out=outp[sname][:, do, sl], in_=y_sb[:])

    nc.compile()
    return nc


def kernel(x, gate_w, gate_b, w1, b1, w2, b2):
    global LAST_EXEC_TIME_NS
    from concourse.bass_utils import run_bass_kernel_spmd

    x = np.asarray(x)
    xf = np.ascontiguousarray(x.reshape(N, D), dtype=np.float32)

    # --- Gate (host, float64 for a stable top-2 selection) ---
    logits = xf.astype(np.float64) @ np.asarray(gate_w).astype(np.float64)
    logits += np.asarray(gate_b).astype(np.float64)
    rows = np.arange(N)
    i1 = np.argmax(logits, axis=1)
    l1 = logits[rows, i1]
    tmp = logits.copy()
    tmp[rows, i1] = -np.inf
    i2 = np.argmax(tmp, axis=1)
    l2 = tmp[rows, i2]
    e2 = np.exp(l2 - l1)          # l1 >= l2
    wa = (1.0 / (1.0 + e2)).astype(np.float32)
    wb = (e2 / (1.0 + e2)).astype(np.float32)

    # --- Dispatch: per-expert token lists ---
    sels, wgts = [], []
    for e in range(E):
        sel = np.where((i1 == e) | (i2 == e))[0]
        wgt = np.where(i1[sel] == e, wa[sel], wb[sel])
        sels.append(sel)
        wgts.append(wgt)
    loads = [len(s) for s in sels]

    # --- Slot cover: segment caps + expert->slot assignment ---
    s1, s2, combos = _solve_slots(loads)
    caps = {"A": s1, "B": s2}

    # slot_expert[seg][core] = expert owning that slot (or -1 = unused)
    # slot_fill[seg][core] = (expert, start_idx, count) token range
    slot_expert = {"A": [-1] * E, "B": [-1] * E}
    slot_fill = {"A": [None] * E, "B": [None] * E}
    next_slot = {"A": 0, "B": 0}
    for e in range(E):
        j, k = combos[e]
        claimed = [("A", next_slot["A"] + i) for i in range(j)] + \
                  [("B", next_slot["B"] + i) for i in range(k)]
        next_slot["A"] += j
        next_slot["B"] += k
        off = 0
        for sname, slot in claimed:
            cnt = min(caps[sname], loads[e] - off)
            slot_expert[sname][slot] = e
            slot_fill[sname][slot] = (e, off, max(cnt, 0))
            off += max(cnt, 0)
        assert off >= loads[e], f"slot cover failed for expert {e}"

    # --- Per-core input maps ---
    w1a = np.asarray(w1, dtype=np.float32)
    b1a = np.asarray(b1, dtype=np.float32)
    w2a = np.asarray(w2, dtype=np.float32)
    b2a = np.asarray(b2, dtype=np.float32)
    wcache = {}

    def expert_weights(e):
        if e not in wcache:
            w1_r = np.ascontiguousarray(
                w1a[e].reshape(KD, P, MH, P).transpose(2, 1, 0, 3)).astype(BF16)
            w2_r = np.ascontiguousarray(
                w2a[e].reshape(MH, P, KD, P).transpose(2, 1, 0, 3)).astype(BF16)
            b1_r = np.ascontiguousarray(b1a[e].reshape(MH, P).T)
            b2_r = np.ascontiguousarray(b2a[e].reshape(KD, P).T)
            wcache[e] = (w1_r, b1_r, w2_r, b2_r)
        return wcache[e]

    in_maps = []
    for c in range(E):
        m = {}
        for sname in ("A", "B"):
            cap = caps[sname]
            fill = slot_fill[sname][c]
            xe = np.zeros((cap, D), dtype=np.float32)
            eid = 0
            if fill is not None:
                eid, off, cnt = fill
                xe[:cnt] = xf[sels[eid][off:off + cnt]]
            xT_r = xe.T.reshape(KD, P, cap).transpose(1, 0, 2).astype(BF16)
            w1_r, b1_r, w2_r, b2_r = expert_weights(eid)
            m[f"w1{sname}"] = w1_r
            m[f"b1{sname}"] = b1_r
            m[f"w2{sname}"] = w2_r
            m[f"b2{sname}"] = b2_r
            for i, sl in enumerate(_chunks(cap)):
                m[f"x{sname}{i}"] = np.ascontiguousarray(xT_r[:, :, sl])
        in_maps.append(m)

    key = (s1, s2)
    if key not in _cached_nc:
        _cached_nc[key] = _build(s1, s2)
    nc = _cached_nc[key]

    trace = os.environ.get("MOE_KERNEL_PROFILE", "0") == "1"
    if trace:
        trace = _ensure_ntff_hook()
    res = None
    for attempt in range(3):
        try:
            res = run_bass_kernel_spmd(nc, in_maps, core_ids=list(range(E)),
                                       trace=trace and attempt == 0)
            break
        except Exception:
            # Device-unrecoverable NRT errors are transient here; retry with
            # a fresh PJRT client (last attempt re-raises).
            if attempt == 2:
                raise
            try:
                import jax
                jax.clear_caches()
                jax._src.api.clear_backends()
            except Exception:
                pass
    LAST_EXEC_TIME_NS = res.exec_time_ns

    # --- Combine (host) ---
    out_acc = np.zeros((N, D), dtype=np.float32)
    for sname in ("A", "B"):
        cap = caps[sname]
        for c in range(E):
            fill = slot_fill[sname][c]
            if fill is None or fill[2] == 0:
                continue
            e, off, cnt = fill
            yT = np.asarray(res.results[c][f"out{sname}"]).astype(np.float32)
            y = yT.transpose(1, 0, 2).reshape(D, cap).T    # [cap, D]
            toks = sels[e][off:off + cnt]
            out_acc[toks] += wgts[e][off:off + cnt, None] * y[:cnt]

    return out_acc.reshape(B, T, D)


# revision 21
# speedup vs baseline: 1.0247x; 1.0012x over previous
"""MoE layer (top-2 of 8 experts) on 8 Trainium2 NeuronCores.

Strategy (self-contained; shapes hardcoded for B=4,T=1024,D=1024,E=8,K=2,H=4096):
  - Host: gate logits + top-2 + softmax (float64 for a stable selection).
  - Slot-cover dispatch: every core runs the SAME kernel with two
    fixed-capacity column segments (s1 >= s2).  Chip-wide that gives 8 slots
    of cap s1 and 8 of cap s2; a small DP assigns each expert a set of slots
    (e.g. a 1101-token expert takes two s1 slots on different cores, a
    1035-token expert takes one s1 + one s2).  This shrinks the per-core
    column count C = s1+s2 from max_e(load_e) (~1101) toward the perfect
    balance sum/8 = 1024 while keeping one compile-time kernel structure.
  - Device, SPMD over 8 cores: per segment a transposed-layout FFN
      hT = gelu(w1.T @ xT + b1)   [H, s]
      yT = w2.T @ hT + b2         [D, s]
    bf16 matmul inputs, f32 PSUM accumulation; outputs DMA'd as bf16.
    Weight tiles are stationary (lhsT); tokens stream as the moving operand,
    so the column capacity per segment is exact (no 128-quantization).
    All chunks are >=256 cols so LDWEIGHTS (~100ns) hides under the stream.
  - Host combine: weighted scatter-add of the slot outputs.
"""

import os

import numpy as np
import ml_dtypes

B, T, D = 4, 1024, 1024
E, K, H = 8, 2, 4 * 1024
N = B * T
P = 128
KD = D // P           # 8  k-tiles in GEMM1 / output d-tiles in GEMM2
MH = H // P           # 32 m-tiles in GEMM1 / k-tiles in GEMM2
BF16 = ml_dtypes.bfloat16
WARMUP_MM = 28

LAST_EXEC_TIME_NS = None
_cached_nc = {}


def _chunks(c):
    """Split a segment's columns into PSUM-bank-sized (<=512 f32) slices.

    Near-equal so every chunk stays >=256 for any c>=256, keeping LDWEIGHTS
    (~117ns) hidden under the matmul stream.
    """
    n = -(-c // 512)
    base = c // n
    rem = c - base * n
    sizes = [base + (1 if i < rem else 0) for i in range(n)]
    out, off = [], 0
    for s in sizes:
        out.append(slice(off, off + s))
        off += s
    return out


def _solve_slots(loads):
    """Pick segment caps (s1, s2) and an expert->slot assignment.

    8 slots of cap s1 and 8 of cap s2 (one of each per core).  Expert e
    claims j s1-slots + k s2-slots with j*s1 + k*s2 >= loads[e].
    Minimizes C = s1 + s2; among ties prefers the largest minimum chunk.
    Returns (s1, s2, combos) with combos[e] = (j, k).
    """
    import time as _time
    loads = list(map(int, loads))
    maxload = max(loads)
    lb = max(512, -(-sum(loads) // 8))
    c0 = 2 * (-(-maxload // 2))          # always-feasible fallback: (1,1) each
    deadline = _time.time() + 15.0

    def feasible(s1, s2):
        # Pareto-minimal combos per expert, then DP over (used1, used2).
        combo_opts = []
        for n_e in loads:
            opts = []
            if n_e == 0:
                opts.append((0, 0))
            else:
                for j in range(0, 9):
                    rest = n_e - j * s1
                    k = 0 if rest <= 0 else -(-rest // s2)
                    if k <= 8:
                        opts.append((j, k))
                        if rest <= 0:
                            break
            if not opts:
                return None
            combo_opts.append(opts)
        states = {(0, 0): []}
        for opts in combo_opts:
            nxt = {}
            for (u1, u2), hist in states.items():
                for (j, k) in opts:
                    v1, v2 = u1 + j, u2 + k
                    if v1 <= 8 and v2 <= 8 and (v1, v2) not in nxt:
                        nxt[(v1, v2)] = hist + [(j, k)]
            if not nxt:
                return None
            states = nxt
        return next(iter(states.values()))

    for C in range(lb, c0 + 1):
        best = None
        if _time.time() > deadline:
            break
        for s1 in range(-(-C // 2), C - 256 + 1):
            s2 = C - s1
            if s2 < 256:
                break
            combos = feasible(s1, s2)
            if combos is None:
                continue
            minchunk = min(min(sl.stop - sl.start for sl in _chunks(s))
                           for s in (s1, s2))
            if best is None or minchunk > best[0]:
                best = (minchunk, s1, s2, combos)
        if best is not None:
            return best[1], best[2], best[3]
    # timed out or exhausted: capacity fallback, always feasible
    s = max(256, -(-maxload // 2))
    return s, s, [(1, 1) for _ in loads]


def _ensure_ntff_hook():
    """Register the axon NTFF profile hook if the image lacks antenv.axon_hooks."""
    import sys
    import types
    try:
        from antenv.axon_hooks import get_axon_ntff_profile_hook
        return get_axon_ntff_profile_hook() is not None
    except ImportError:
        pass
    try:
        import antenv
        from trn_agent_boot.trn_boot import _ntff_profile_via_ctypes
        mod = types.ModuleType("antenv.axon_hooks")
        holder = [None]
        mod.set_axon_ntff_profile_hook = lambda h: holder.__setitem__(0, h)
        mod.get_axon_ntff_profile_hook = lambda: holder[0]
        sys.modules["antenv.axon_hooks"] = mod
        antenv.axon_hooks = mod
        mod.set_axon_ntff_profile_hook(
            _ntff_profile_via_ctypes("/opt/axon/libaxon_pjrt.so"))
        return True
    except Exception:
        return False


def _build(s1, s2):
    import concourse.mybir as mybir
    import concourse.tile as tile
    from concourse import bacc

    nc = bacc.Bacc(None, target_bir_lowering=False)

    segs = [("A", s1), ("B", s2)]
    xs = {}
    w1p = {}
    b1p = {}
    w2p = {}
    b2p = {}
    outp = {}
    for sname, cap in segs:
        xs[sname] = [nc.declare_dram_parameter(
            f"x{sname}{i}", [P, KD, sl.stop - sl.start],
            mybir.dt.bfloat16, isOutput=False)
            for i, sl in enumerate(_chunks(cap))]
        w1p[sname] = nc.declare_dram_parameter(
            f"w1{sname}", [MH, P, KD, P], mybir.dt.bfloat16, isOutput=False)
        b1p[sname] = nc.declare_dram_parameter(
            f"b1{sname}", [P, MH], mybir.dt.float32, isOutput=False)
        w2p[sname] = nc.declare_dram_parameter(
            f"w2{sname}", [KD, P, MH, P], mybir.dt.bfloat16, isOutput=False)
        b2p[sname] = nc.declare_dram_parameter(
            f"b2{sname}", [P, KD], mybir.dt.float32, isOutput=False)
        outp[sname] = nc.declare_dram_parameter(
            f"out{sname}", [P, KD, cap], mybir.dt.bfloat16, isOutput=True)

    GELU = mybir.ActivationFunctionType.Gelu

    with tile.TileContext(nc) as tc, \
         tc.tile_pool(name="singles", bufs=1) as singles, \
         tc.tile_pool(name="w1poolA", bufs=5) as w1poolA, \
         tc.tile_pool(name="w1poolB", bufs=5) as w1poolB, \
         tc.tile_pool(name="w2poolA", bufs=2) as w2poolA, \
         tc.tile_pool(name="w2poolB", bufs=2) as w2poolB, \
         tc.tile_pool(name="ypool", bufs=3) as ypool, \
         tc.tile_pool(name="psum", bufs=4, space="PSUM") as psum_pool:

        w1pool = {"A": w1poolA, "B": w1poolB}
        w2pool = {"A": w2poolA, "B": w2poolB}

        # PE warm-up: dependency-free matmuls keep the TensorEngine busy
        # through the DMA ring-init + first-data window and release the HAM
        # clock gate, so the PE is warm when the real stream starts.
        warm_sb = singles.tile([P, 2 * P], mybir.dt.bfloat16)
        nc.vector.memset(warm_sb[:], 0.0)
        ps_warm = psum_pool.tile([P, 2 * P], mybir.dt.float32, name="ps_warm",
                                 tag="ps1")
        for _ in range(WARMUP_MM):
            nc.tensor.matmul(ps_warm[:], warm_sb[:, :P], warm_sb[:],
                             start=True, stop=True)

        # Segment-major schedule: all of A's GEMM1 first, then B's.  Only
        # segment A's x is on the startup critical path; B's inputs stream
        # in during A's ~30us of compute (sync queue drains in issue order,
        # so B's loads queue behind A's critical-path data).
        x_sbs = {}
        b1_sb = {}
        b2_sb = {}
        h_sb = {}
        for sname, cap in segs:
            h_sb[sname] = singles.tile([P, MH, cap], mybir.dt.bfloat16,
                                        name=f"h{sname}_sb")

        # GEMM1: hT[mo*128+p, c] = gelu(sum_k w1[k,:].T @ xT[k,:] + b1)
        NPRE = 4
        for sname, cap in segs:
            # interleave the first w1 tiles with the x chunks in issue order
            # so neither stream starves while the DMA rings ramp up
            pre = []
            w1_t0 = w1pool[sname].tile([P, KD, P], mybir.dt.bfloat16,
                                       name=f"w1{sname}_t")
            nc.sync.dma_start(out=w1_t0[:], in_=w1p[sname][0])
            pre.append(w1_t0)
            chs = _chunks(cap)
            x_sbs[sname] = []
            b1_sb[sname] = singles.tile([P, MH], mybir.dt.float32,
                                         name=f"b1{sname}_sb")
            b2_sb[sname] = singles.tile([P, KD], mybir.dt.float32,
                                         name=f"b2{sname}_sb")
            for i, sl in enumerate(chs):
                x_sb = singles.tile([P, KD, sl.stop - sl.start],
                                    mybir.dt.bfloat16, name=f"x{sname}{i}")
                nc.sync.dma_start(out=x_sb[:], in_=xs[sname][i][:])
                x_sbs[sname].append(x_sb)
                if len(pre) <= NPRE:
                    w1_tn = w1pool[sname].tile([P, KD, P], mybir.dt.bfloat16,
                                               name=f"w1{sname}_t")
                    nc.sync.dma_start(out=w1_tn[:], in_=w1p[sname][len(pre)])
                    pre.append(w1_tn)
            nc.sync.dma_start(out=b1_sb[sname][:], in_=b1p[sname][:])
            nc.sync.dma_start(out=b2_sb[sname][:], in_=b2p[sname][:])
            while len(pre) < NPRE:
                w1_tn = w1pool[sname].tile([P, KD, P], mybir.dt.bfloat16,
                                           name=f"w1{sname}_t")
                nc.sync.dma_start(out=w1_tn[:], in_=w1p[sname][len(pre)])
                pre.append(w1_tn)
            for mo in range(MH):
                if mo < len(pre):
                    w1_t = pre[mo]
                else:
                    w1_t = w1pool[sname].tile([P, KD, P], mybir.dt.bfloat16,
                                              name=f"w1{sname}_t")
                    nc.sync.dma_start(out=w1_t[:], in_=w1p[sname][mo])
                for si, sl in enumerate(_chunks(cap)):
                    ps1 = psum_pool.tile([P, sl.stop - sl.start],
                                         mybir.dt.float32, name="ps1")
                    for k in range(KD):
                        nc.tensor.matmul(ps1[:], w1_t[:, k, :],
                                         x_sbs[sname][si][:, k, :],
                                         start=(k == 0), stop=(k == KD - 1))
                    nc.scalar.activation(h_sb[sname][:, mo, sl], ps1[:], GELU,
                                         bias=b1_sb[sname][:, mo:mo + 1])

        # GEMM2: yT[do*128+p, c] = sum_k w2[k,:].T @ hT[k,:] + b2
        # B first so the kernel tail drains on A's smaller last chunk.
        for sname, cap in segs[::-1]:
            for do in range(KD):
                w2_t = w2pool[sname].tile([P, MH, P], mybir.dt.bfloat16,
                                          name=f"w2{sname}_t")
                nc.sync.dma_start(out=w2_t[:], in_=w2p[sname][do])
                for sl in _chunks(cap):
                    ps2 = psum_pool.tile([P, sl.stop - sl.start],
                                         mybir.dt.float32, name="ps2")
                    for k in range(MH):
                        nc.tensor.matmul(ps2[:], w2_t[:, k, :],
                                         h_sb[sname][:, k, sl],
                                         start=(k == 0), stop=(k == MH - 1))
                    y_sb = ypool.tile([P, sl.stop - sl.start],
                                      mybir.dt.bfloat16, name="y_sb")
                    nc.vector.tensor_scalar_add(y_sb[:], ps2[:],
                                                b2_sb[sname][:, do:do + 1])
                    nc.sync.dma_start(out=outp[sname][:, do, sl], in_=y_sb[:])

    nc.compile()
    return nc


def kernel(x, gate_w, gate_b, w1, b1, w2, b2):
    global LAST_EXEC_TIME_NS
    from concourse.bass_utils import run_bass_kernel_spmd

    x = np.asarray(x)
    xf = np.ascontiguousarray(x.reshape(N, D), dtype=np.float32)

    # --- Gate (host, float64 for a stable top-2 selection) ---
    logits = xf.astype(np.float64) @ np.asarray(gate_w).astype(np.float64)
    logits += np.asarray(gate_b).astype(np.float64)
    rows = np.arange(N)
    i1 = np.argmax(logits, axis=1)
    l1 = logits[rows, i1]
    tmp = logits.copy()
    tmp[rows, i1] = -np.inf
    i2 = np.argmax(tmp, axis=1)
    l2 = tmp[rows, i2]
    e2 = np.exp(l2 - l1)          # l1 >= l2
    wa = (1.0 / (1.0 + e2)).astype(np.float32)
    wb = (e2 / (1.0 + e2)).astype(np.float32)

    # --- Dispatch: per-expert token lists ---
    sels, wgts = [], []
    for e in range(E):
        sel = np.where((i1 == e) | (i2 == e))[0]
        wgt = np.where(i1[sel] == e, wa[sel], wb[sel])
        sels.append(sel)
        wgts.append(wgt)
    loads = [len(s) for s in sels]

    # --- Slot cover: segment caps + expert->slot assignment ---
    s1, s2, combos = _solve_slots(loads)
    caps = {"A": s1, "B": s2}

    # slot_expert[seg][core] = expert owning that slot (or -1 = unused)
    # slot_fill[seg][core] = (expert, start_idx, count) token range
    slot_expert = {"A": [-1] * E, "B": [-1] * E}
    slot_fill = {"A": [None] * E, "B": [None] * E}
    next_slot = {"A": 0, "B": 0}
    for e in range(E):
        j, k = combos[e]
        claimed = [("A", next_slot["A"] + i) for i in range(j)] + \
                  [("B", next_slot["B"] + i) for i in range(k)]
        next_slot["A"] += j
        next_slot["B"] += k
        off = 0
        for sname, slot in claimed:
            cnt = min(caps[sname], loads[e] - off)
            slot_expert[sname][slot] = e
            slot_fill[sname][slot] = (e, off, max(cnt, 0))
            off += max(cnt, 0)
        assert off >= loads[e], f"slot cover failed for expert {e}"

    # --- Per-core input maps ---
    w1a = np.asarray(w1, dtype=np.float32)
    b1a = np.asarray(b1, dtype=np.float32)
    w2a = np.asarray(w2, dtype=np.float32)
    b2a = np.asarray(b2, dtype=np.float32)
    wcache = {}

    def expert_weights(e):
        if e not in wcache:
            w1_r = np.ascontiguousarray(
                w1a[e].reshape(KD, P, MH, P).transpose(2, 1, 0, 3)).astype(BF16)
            w2_r = np.ascontiguousarray(
                w2a[e].reshape(MH, P, KD, P).transpose(2, 1, 0, 3)).astype(BF16)
            b1_r = np.ascontiguousarray(b1a[e].reshape(MH, P).T)
            b2_r = np.ascontiguousarray(b2a[e].reshape(KD, P).T)
            wcache[e] = (w1_r, b1_r, w2_r, b2_r)
        return wcache[e]

    in_maps = []
    for c in range(E):
        m = {}
        for sname in ("A", "B"):
            cap = caps[sname]
            fill = slot_fill[sname][c]
            xe = np.zeros((cap, D), dtype=np.float32)
            eid = 0
            if fill is not None:
                eid, off, cnt = fill
                xe[:cnt] = xf[sels[eid][off:off + cnt]]
            xT_r = xe.T.reshape(KD, P, cap).transpose(1, 0, 2).astype(BF16)
            w1_r, b1_r, w2_r, b2_r = expert_weights(eid)
            m[f"w1{sname}"] = w1_r
            m[f"b1{sname}"] = b1_r
            m[f"w2{sname}"] = w2_r
            m[f"b2{sname}"] = b2_r
            for i, sl in enumerate(_chunks(cap)):
                m[f"x{sname}{i}"] = np.ascontiguousarray(xT_r[:, :, sl])
        in_maps.append(m)

    key = (s1, s2)
    if key not in _cached_nc:
        _cached_nc[key] = _build(s1, s2)
    nc = _cached_nc[key]

    trace = os.environ.get("MOE_KERNEL_PROFILE", "0") == "1"
    if trace:
        trace = _ensure_ntff_hook()
    res = None
    for attempt in range(3):
        try:
            res = run_bass_kernel_spmd(nc, in_maps, core_ids=list(range(E)),
                                       trace=trace and attempt == 0)
            break
        except Exception:
            # Device-unrecoverable NRT errors are transient here; retry with
            # a fresh PJRT client (last attempt re-raises).
            if attempt == 2:
                raise
            try:
                import jax
                jax.clear_caches()
                jax._src.api.clear_backends()
            except Exception:
                pass
    LAST_EXEC_TIME_NS = res.exec_time_ns

    # --- Combine (host) ---
    out_acc = np.zeros((N, D), dtype=np.float32)
    for sname in ("A", "B"):
        cap = caps[sname]
        for c in range(E):
            fill = slot_fill[sname][c]
            if fill is None or fill[2] == 0:
                continue
            e, off, cnt = fill
            yT = np.asarray(res.results[c][f"out{sname}"]).astype(np.float32)
            y = yT.transpose(1, 0, 2).reshape(D, cap).T    # [cap, D]
            toks = sels[e][off:off + cnt]
            out_acc[toks] += wgts[e][off:off + cnt, None] * y[:cnt]

    return out_acc.reshape(B, T, D)
